# revision 42
# baseline (speedup 1.0000x reference)
"""Deformable-Conv (DCNv1) + SyncBN + LeakyReLU Trainium2 kernel, v2.

Self-contained: shards the full inputs over 8 NeuronCores (data-parallel over
(batch, row-half); BN stats all-reduced on-device), runs one SPMD Bass/Tile
kernel via run_bass_kernel_spmd, and reassembles the full output.

v2 dataflow (per core, 64 output rows x 128 cols, 256 out-ch):
  HOST  build xpt2: "4-corner unit" table in DRAM. Unit q holds the fp16
        channel vectors of padded positions q and q+130 (vertical pair).
        One 1KB gather element = units (q, q+1) = all four bilinear corners.
  P1    offset conv (9 accumulating fp16 matmuls per 4-row group) + PE
        transposes -> offsets in [w-partition, (row, tap)] layout.
  P2    elementwise chain (DVE fp32): bilinear indices + border-folded weights.
  P3    wrapped int16 index build for dma_gather (double PE-transpose trick),
        single index stream (one descriptor per position*tap).
  P4    per (8-row group, tap): one dma_gather (1024 idx, 1KB elems; fits the
        1024-descriptor SWDGE ring); per (tap, row) the 4-term bilinear
        combine runs as
          t = wlt*LT (DVE ts in 4x perf mode)
          psum += T(t) + RT^T*diag(wrt) + LB^T*diag(wlb) + RB^T*diag(wrb)
        where the diag(w) matrices are built by DVE 4x tensor_scalar on the
        identity and the four PE matmuls accumulate scale+transpose+sum in
        one PSUM group. Produce (DVE) runs two taps ahead of consume (PE).
        ACT evicts psum -> fp16 rhs; fp16 matmuls accumulate over taps in
        PSUM; ACT copy-with-accum + Square for BN stats.
  P5    BN stats AllReduce (2x256 floats) + scale/bias solve.
  P6    fused affine+LeakyReLU via max(a, 0.1*a) (DVE) + fp16 store.
"""
import sys

sys.path.insert(0, "/opt/trn_rl_repo")

import numpy as np

import concourse.bacc as bacc
import concourse.mybir as mybir
from concourse import tile
from concourse.ap import AP

ALU = mybir.AluOpType
DT = mybir.dt
AF = mybir.ActivationFunctionType

N_CORES = 8
B, C, O, H, W = 4, 128, 256, 128, 128
KS, NT = 3, 9
ROWS = 64                 # output rows per core
NG2, R8 = 8, 8            # main loop: 8 groups of 8 rows
Hp = H + 2                # 130
NUNITS = 17152            # 130*130 = 16900 units + zero tail
NPOS = ROWS * W           # 8192
EPS = 1e-5
LEAK = 0.1
MAGIC = float(3 << 22)    # 1.5 * 2^23: fp32 round-to-int magic
CH = ROWS * NT            # 576: elementwise-chain free size
CW = NG2 * NT * R8        # 576: index-matrix columns (one per (g2, tap, row))

DX = np.repeat(np.arange(-1, 2), 3).astype(np.float32)
DY = np.tile(np.arange(-1, 2), 3).astype(np.float32)


def build_kernel(with_collective=True):
    nc = bacc.Bacc("TRN2", target_bir_lowering=False)

    # ---- I/O ----
    xpt2_d = nc.dram_tensor("xpt2", [NUNITS * 2 * C], DT.float16,
                            kind="ExternalInput")
    xwin_d = nc.dram_tensor("xwin", [C, 66 * Hp], DT.float16,
                            kind="ExternalInput")
    pwT_d = nc.dram_tensor("pwT", [NT, C, 2 * NT], DT.float16,
                           kind="ExternalInput")
    wT_d = nc.dram_tensor("wT", [NT, C, O], DT.float16, kind="ExternalInput")
    ax_d = nc.dram_tensor("Ax", [128, CH], DT.float32, kind="ExternalInput")
    by_d = nc.dram_tensor("By", [128, CH], DT.float32, kind="ExternalInput")
    gam_d = nc.dram_tensor("gamma2", [128, 2], DT.float32, kind="ExternalInput")
    bet_d = nc.dram_tensor("beta2", [128, 2], DT.float32, kind="ExternalInput")
    idf_d = nc.dram_tensor("identf", [128, 128], DT.float32,
                           kind="ExternalInput")
    idh_d = nc.dram_tensor("identh", [128, 128], DT.float16,
                           kind="ExternalInput")

    out_d = nc.dram_tensor("out", [2, 128, NPOS], DT.float16,
                           kind="ExternalOutput")

    cc_in = nc.dram_tensor("cc_in", [128, 4], DT.float32)
    cc_out = nc.dram_tensor("cc_out", [128, 4], DT.float32)

    taps = [(ky, kx) for ky in range(3) for kx in range(3)]

    with tile.TileContext(nc) as tc:
        with tc.tile_pool(name="pp", bufs=1) as pp, \
             tc.tile_pool(name="pbig", bufs=1) as pbig, \
             tc.tile_pool(name="pch", bufs=10) as pch, \
             tc.tile_pool(name="po", bufs=2) as po, \
             tc.tile_pool(name="pw4", bufs=1) as pw4, \
             tc.tile_pool(name="pg", bufs=6) as pg, \
             tc.tile_pool(name="pst", bufs=3) as pst, \
             tc.tile_pool(name="pdg", bufs=14) as pdg, \
             tc.tile_pool(name="prh", bufs=3) as prh, \
             tc.tile_pool(name="pp6", bufs=2) as pp6, \
             tc.tile_pool(name="pps", bufs=2, space="PSUM") as pps, \
             tc.tile_pool(name="ppre", bufs=1, space="PSUM") as ppre, \
             tc.tile_pool(name="ppacc", bufs=2, space="PSUM") as ppacc:

            # ---------------- constants ----------------
            pw_sb = pp.tile([C, NT * 2 * NT], DT.float16, tag="pw")
            nc.sync.dma_start(pw_sb[:].rearrange("c (t m) -> c t m", m=2 * NT),
                              pwT_d[:].transpose([1, 0, 2]))
            wt_sb = pp.tile([C, NT * O], DT.float16, tag="wt")
            nc.sync.dma_start(wt_sb[:].rearrange("c (t o) -> c t o", o=O),
                              wT_d[:].transpose([1, 0, 2]))
            ax_sb = pp.tile([128, CH], DT.float32, tag="ax")
            nc.sync.dma_start(ax_sb[:], ax_d[:])
            by_sb = pp.tile([128, CH], DT.float32, tag="by")
            nc.sync.dma_start(by_sb[:], by_d[:])
            gam_sb = pp.tile([128, 2], DT.float32, tag="gam")
            nc.sync.dma_start(gam_sb[:], gam_d[:])
            bet_sb = pp.tile([128, 2], DT.float32, tag="bet")
            nc.sync.dma_start(bet_sb[:], bet_d[:])
            idf = pp.tile([128, 128], DT.float32, tag="idf")
            nc.sync.dma_start(idf[:], idf_d[:])
            idh = pp.tile([128, 128], DT.float16, tag="idh")
            nc.sync.dma_start(idh[:], idh_d[:])

            xwin = pbig.tile([C, 66 * Hp], DT.float16, tag="xwin")
            nc.sync.dma_start(xwin[:], xwin_d[:])

            # PE p-state warmup: keep the tensor engine continuously busy so
            # the offset conv runs at full clock.
            wrm = ppre.tile([128, 128], DT.float16, tag="ptr", name="wrm")
            for _ in range(24):
                nc.tensor.transpose(out=wrm[:], in_=idh[:], identity=idh[:])

            # ---------------- P1-P4: pipelined per g2-pair -------------------
            # Prologue work (offset conv, chain, wrapped-index build) is
            # emitted per pair of 8-row groups so the first gather issues
            # after ~1/4 of the prologue; later pairs overlap the gathers.
            offT = pw4.tile([128, ROWS * 2 * NT], DT.float32, tag="offT")
            pwr = pw_sb[:].rearrange("c (t m) -> c t m", m=2 * NT)
            offv = offT[:].rearrange("p (r m) -> p r m", m=2 * NT)
            wlt = pw4.tile([128, CH], DT.float32, tag="wlt")
            wlb = pw4.tile([128, CH], DT.float32, tag="wlb")
            wrt = pw4.tile([128, CH], DT.float32, tag="wrt")
            wrb = pw4.tile([128, CH], DT.float32, tag="wrb")
            cmat = pw4.tile([128, CW], DT.float32, tag="cmat")
            wrap = pw4.tile([128, CW * 8], DT.int16, tag="wrap")

            PC = 144  # chain/index columns per pair (16 rows x 9 taps)

            def emit_pair(k):
                # --- offset conv for rows 16k .. 16k+16 ---
                for g2l in range(2):
                    g2 = 2 * k + g2l
                    ps_tr = ppre.tile([128, PC], DT.float32, tag="ptr",
                                      name="ps_tr")
                    for q in range(2):
                        r0 = g2 * 8 + q * 4
                        ps_off = ppre.tile([2 * NT, 512], DT.float32,
                                           tag="poff", name="ps_off")
                        for t, (ky, kx) in enumerate(taps):
                            base = (r0 + ky) * Hp + kx
                            rhs = AP(xwin.tensor, xwin[:].offset + base,
                                     [xwin[:].ap[0], [Hp, 4], [1, W]])
                            nc.tensor.matmul(ps_off[:], lhsT=pwr[:, t],
                                             rhs=rhs,
                                             start=(t == 0), stop=(t == 8))
                        offc = po.tile([2 * NT, 512], DT.float32, tag="offc",
                                        name="offc")
                        nc.scalar.copy(offc[:], ps_off[:])
                        for r4 in range(4):
                            osl = slice((q * 4 + r4) * 2 * NT,
                                        (q * 4 + r4 + 1) * 2 * NT)
                            nc.tensor.transpose(
                                out=ps_tr[:, osl],
                                in_=offc[:, r4 * 128:(r4 + 1) * 128],
                                identity=idf[:2 * NT, :2 * NT])
                    nc.vector.tensor_copy(
                        offT[:, g2 * PC:(g2 + 1) * PC], ps_tr[:])

                # --- elementwise chain for this pair ---
                cs = slice(k * PC, (k + 1) * PC)

                def cht():
                    return pch.tile([128, PC], DT.float32, tag="ch",
                                    name="cht")

                ox = cht()
                nc.vector.tensor_copy(
                    ox[:].rearrange("p (r n) -> p r n", n=NT),
                    offv[:, 16 * k:16 * k + 16, 0:NT])
                oy = cht()
                nc.vector.tensor_copy(
                    oy[:].rearrange("p (r n) -> p r n", n=NT),
                    offv[:, 16 * k:16 * k + 16, NT:2 * NT])
                px = cht()
                nc.vector.tensor_tensor(out=px[:], in0=ox[:],
                                        in1=ax_sb[:, cs], op=ALU.add)
                py = cht()
                nc.vector.tensor_tensor(out=py[:], in0=oy[:],
                                        in1=by_sb[:, cs], op=ALU.add)

                def floor_(v):
                    fl = cht()
                    nc.vector.tensor_scalar(out=fl[:], in0=v[:],
                                            scalar1=MAGIC, scalar2=MAGIC,
                                            op0=ALU.add, op1=ALU.subtract)
                    g_ = cht()
                    nc.vector.tensor_tensor(out=g_[:], in0=fl[:], in1=v[:],
                                            op=ALU.is_gt)
                    nc.vector.tensor_tensor(out=fl[:], in0=fl[:], in1=g_[:],
                                            op=ALU.subtract)
                    return fl

                fx = floor_(px)
                fy = floor_(py)

                def clip_lo_hi(v):
                    q0 = cht()
                    nc.vector.tensor_scalar(out=q0[:], in0=v[:], scalar1=0.0,
                                            scalar2=129.0, op0=ALU.max,
                                            op1=ALU.min)
                    q1 = cht()
                    nc.vector.tensor_scalar(out=q1[:], in0=v[:], scalar1=-1.0,
                                            scalar2=1.0, op0=ALU.max,
                                            op1=ALU.add)
                    nc.vector.tensor_scalar(out=q1[:], in0=q1[:],
                                            scalar1=129.0, scalar2=None,
                                            op0=ALU.min)
                    return q0, q1

                qltx, qrbx = clip_lo_hi(fx)
                qlty, qrby = clip_lo_hi(fy)
                pcx = cht()
                nc.vector.tensor_scalar(out=pcx[:], in0=px[:], scalar1=0.0,
                                        scalar2=129.0, op0=ALU.max,
                                        op1=ALU.min)
                pcy = cht()
                nc.vector.tensor_scalar(out=pcy[:], in0=py[:], scalar1=0.0,
                                        scalar2=129.0, op0=ALU.max,
                                        op1=ALU.min)

                def weights(qlt, qrb, pc):
                    a0 = cht()
                    nc.vector.scalar_tensor_tensor(out=a0[:], in0=qlt[:],
                                                   scalar=1.0, in1=pc[:],
                                                   op0=ALU.add,
                                                   op1=ALU.subtract)
                    a1 = cht()
                    nc.vector.scalar_tensor_tensor(out=a1[:], in0=pc[:],
                                                   scalar=1.0, in1=qrb[:],
                                                   op0=ALU.add,
                                                   op1=ALU.subtract)
                    eq = cht()
                    nc.vector.tensor_tensor(out=eq[:], in0=qrb[:],
                                            in1=qlt[:], op=ALU.is_equal)
                    t = cht()
                    nc.vector.tensor_tensor(out=t[:], in0=eq[:], in1=a1[:],
                                            op=ALU.mult)
                    nc.vector.tensor_tensor(out=a0[:], in0=a0[:], in1=t[:],
                                            op=ALU.add)
                    nc.vector.tensor_scalar(out=eq[:], in0=eq[:],
                                            scalar1=-1.0, scalar2=1.0,
                                            op0=ALU.mult, op1=ALU.add)
                    nc.vector.tensor_tensor(out=a1[:], in0=a1[:], in1=eq[:],
                                            op=ALU.mult)
                    return a0, a1

                a0, a1 = weights(qltx, qrbx, pcx)
                b0, b1 = weights(qlty, qrby, pcy)

                nc.vector.tensor_tensor(out=wlt[:, cs], in0=a0[:], in1=b0[:],
                                        op=ALU.mult)
                nc.vector.tensor_tensor(out=wlb[:, cs], in0=a0[:], in1=b1[:],
                                        op=ALU.mult)
                nc.vector.tensor_tensor(out=wrt[:, cs], in0=a1[:], in1=b0[:],
                                        op=ALU.mult)
                nc.vector.tensor_tensor(out=wrb[:, cs], in0=a1[:], in1=b1[:],
                                        op=ALU.mult)

                idx0 = cht()
                nc.vector.scalar_tensor_tensor(out=idx0[:], in0=qltx[:],
                                               scalar=130.0, in1=qlty[:],
                                               op0=ALU.mult, op1=ALU.add)

                cview = cmat[:, cs].rearrange("p (g n j) -> p g n j",
                                              g=2, n=NT)
                sview = idx0[:].rearrange("p (g j n) -> p g j n", g=2, j=R8)
                nc.vector.tensor_copy(cview, sview.transpose([0, 1, 3, 2]))

                # --- wrapped int16 index build for this pair ---
                for lo_l, hi_l in ((0, 128), (128, PC)):
                    lo = k * PC + lo_l
                    cksz = hi_l - lo_l
                    ps = ppre.tile([128, 128], DT.float32, tag="ptr",
                                   name="psT2")
                    nc.tensor.transpose(out=ps[:cksz, :],
                                        in_=cmat[:, lo:lo + cksz],
                                        identity=idf[:])
                    tsb = po.tile([128, 128], DT.float32, tag="tsb",
                                   name="tsb")
                    nc.scalar.copy(tsb[:cksz, :], ps[:cksz, :])
                    for a in range(8):
                        wa = ppre.tile([16, 128], DT.float32, tag="ptr",
                                       name="wa")
                        nc.tensor.transpose(
                            out=wa[:, :cksz],
                            in_=tsb[:cksz, 16 * a:16 * a + 16],
                            identity=idf[:cksz, :cksz])
                        dstv = AP(wrap.tensor, wrap[:].offset
                                  + (lo // 16) * 128 + a,
                                  [[wrap[:].ap[0][0], 16],
                                   [128, cksz // 16], [8, 16]])
                        nc.vector.tensor_copy(
                            dstv,
                            wa[:, :cksz].rearrange("p (c j) -> p c j", j=16))
                for cgrp in range(1, 8):
                    wsl = slice(k * PC * 8, (k + 1) * PC * 8)
                    nc.gpsimd.dma_start(
                        out=wrap[16 * cgrp:16 * (cgrp + 1), wsl],
                        in_=wrap[0:16, wsl])

            # ---------------- P4: gather + combine + matmul ------------------
            src_ap = AP(xpt2_d, 0, [[2 * C, NUNITS - 1], [1, 4 * C]])
            out_sb = pbig.tile([128, 2 * NPOS], DT.float16, tag="osb")
            sums = pp.tile([128, 16], DT.float32, tag="sums")
            sqs = pp.tile([128, 16], DT.float32, tag="sqs")
            junk = pp.tile([128, R8 * W], DT.float32, tag="junk")
            wtv = wt_sb[:].rearrange("c (t o) -> c t o", o=O)

            emit_pair(0)
            for g2 in range(NG2):
                pacc = [ppacc.tile([128, R8 * W], DT.float32, tag="acc",
                                   name=f"pacc{i}") for i in range(2)]
                g_ts = []
                for n in range(NT):
                    g_t = pg.tile([128, 8, 4 * C], DT.float16, tag="gt",
                                  name="g_t")
                    fbase = g2 * CW + n * 64
                    nc.gpsimd.dma_gather(
                        out_ap=g_t[:], in_ap=src_ap,
                        idxs_ap=wrap[:, fbase:fbase + 64],
                        num_idxs=8 * 128, num_idxs_reg=8 * 128,
                        elem_size=4 * C, elem_step=2 * C)
                    g_ts.append(g_t)

                def produce(n):
                    g_t = g_ts[n]
                    ta = pst.tile([128, R8 * W], DT.float16, tag="ta")
                    diags = []
                    # diag builds first: they depend only on the weights, so
                    # DVE keeps working while the gather for tap n lands.
                    for jj in range(R8):
                        col = (g2 * R8 + jj) * NT + n
                        drt = pdg.tile([128, 128], DT.float16, tag="drt",
                                       name="drt")
                        nc.vector.tensor_scalar(
                            out=drt[:], in0=idh[:],
                            scalar1=wrt[:, col:col + 1], scalar2=None,
                            op0=ALU.mult)
                        dlb = pdg.tile([128, 128], DT.float16, tag="dlb",
                                       name="dlb")
                        if (jj + n) % 2 == 0:
                            nc.vector.tensor_scalar(
                                out=dlb[:], in0=idh[:],
                                scalar1=wlb[:, col:col + 1], scalar2=None,
                                op0=ALU.mult)
                        else:
                            nc.scalar.activation(
                                out=dlb[:], in_=idh[:], func=AF.Copy,
                                scale=wlb[:, col:col + 1])
                        drb = pdg.tile([128, 128], DT.float16, tag="drb",
                                       name="drb")
                        nc.vector.tensor_scalar(
                            out=drb[:], in0=idh[:],
                            scalar1=wrb[:, col:col + 1],
                            scalar2=None, op0=ALU.mult)
                        diags.append((drt, dlb, drb))
                    for jj in range(R8):
                        col = (g2 * R8 + jj) * NT + n
                        sl = slice(jj * 128, (jj + 1) * 128)
                        nc.vector.tensor_scalar(
                            out=ta[:, sl], in0=g_t[:, jj, 0:C],
                            scalar1=wlt[:, col:col + 1], scalar2=None,
                            op0=ALU.mult)
                    return (g_t, n, ta, diags)

                def consume(st):
                    g_t, n, ta, diags = st
                    rhs16 = prh.tile([128, R8 * W], DT.float16, tag="rhs16")
                    for hh4 in range(2):
                        ps_t = pps.tile([128, 512], DT.float32, tag="tp",
                                        name="ps_t")
                        for jj in range(hh4 * 4, hh4 * 4 + 4):
                            ck = jj
                            sl = slice(jj * 128, (jj + 1) * 128)
                            psl = slice((jj % 4) * 128, (jj % 4 + 1) * 128)
                            drt, dlb, drb = diags[jj]
                            nc.tensor.matmul(ps_t[:, psl], lhsT=ta[:, sl],
                                             rhs=idh[:],
                                             start=True, stop=False)
                            nc.tensor.matmul(ps_t[:, psl],
                                             lhsT=g_t[:, ck, C:2 * C],
                                             rhs=drt[:], start=False,
                                             stop=False)
                            nc.tensor.matmul(ps_t[:, psl],
                                             lhsT=g_t[:, ck, 2 * C:3 * C],
                                             rhs=dlb[:], start=False,
                                             stop=False)
                            nc.tensor.matmul(ps_t[:, psl],
                                             lhsT=g_t[:, ck, 3 * C:4 * C],
                                             rhs=drb[:], start=False,
                                             stop=True)
                        hsl = slice(hh4 * 512, (hh4 + 1) * 512)
                        nc.scalar.copy(rhs16[:, hsl], ps_t[:])
                    for oc in range(2):
                        for hh in range(2):
                            sl = slice(hh * 512, (hh + 1) * 512)
                            nc.tensor.matmul(
                                pacc[oc][:, sl],
                                lhsT=wtv[:, n, oc * 128:(oc + 1) * 128],
                                rhs=rhs16[:, sl],
                                start=(n == 0), stop=(n == 8))

                pend = []
                for n in range(NT):
                    pend.append(produce(n))
                    if len(pend) >= 4:
                        consume(pend.pop(0))
                if g2 == 0:
                    emit_pair(1)
                if g2 == 2:
                    emit_pair(2)
                if g2 == 4:
                    emit_pair(3)
                while pend:
                    consume(pend.pop(0))
                for oc in range(2):
                    seg = slice(oc * NPOS + g2 * R8 * W,
                                oc * NPOS + (g2 + 1) * R8 * W)
                    nc.scalar.activation(
                        out=out_sb[:, seg], in_=pacc[oc][:], func=AF.Copy,
                        accum_out=sums[:, oc * 8 + g2:oc * 8 + g2 + 1])
                    nc.scalar.activation(
                        out=junk[:], in_=pacc[oc][:], func=AF.Square,
                        accum_out=sqs[:, oc * 8 + g2:oc * 8 + g2 + 1])

            # ---------------- P5: BN stats + collective ----------------------
            stats = pp.tile([128, 4], DT.float32, tag="stats")
            nc.vector.tensor_reduce(out=stats[:, 0:1], in_=sums[:, 0:8],
                                    axis=mybir.AxisListType.X, op=ALU.add)
            nc.vector.tensor_reduce(out=stats[:, 1:2], in_=sqs[:, 0:8],
                                    axis=mybir.AxisListType.X, op=ALU.add)
            nc.vector.tensor_reduce(out=stats[:, 2:3], in_=sums[:, 8:16],
                                    axis=mybir.AxisListType.X, op=ALU.add)
            nc.vector.tensor_reduce(out=stats[:, 3:4], in_=sqs[:, 8:16],
                                    axis=mybir.AxisListType.X, op=ALU.add)
            d1 = nc.sync.dma_start(out=cc_in[:], in_=stats[:])
            from concourse.tile_rust import add_dep_helper
            if with_collective:
                cci = nc.gpsimd.collective_compute(
                    "AllReduce", ALU.add,
                    replica_groups=[list(range(N_CORES))],
                    ins=[cc_in[:].opt()], outs=[cc_out[:].opt()])
            else:
                cci = nc.sync.dma_start(out=cc_out[:], in_=cc_in[:])
            add_dep_helper(cci.ins, d1.ins, sync=True, reason="cc after stats")
            ast = pp.tile([128, 4], DT.float32, tag="ast")
            d2 = nc.sync.dma_start(out=ast[:], in_=cc_out[:])
            add_dep_helper(d2.ins, cci.ins, sync=True, reason="readback")

            astv = ast[:].rearrange("p (a b) -> p a b", b=2)
            cnt = float(B * H * W)
            mean = pp.tile([128, 2], DT.float32, tag="mean")
            nc.vector.tensor_scalar(out=mean[:], in0=astv[:, :, 0],
                                    scalar1=1.0 / cnt, scalar2=None,
                                    op0=ALU.mult)
            var = pp.tile([128, 2], DT.float32, tag="var")
            nc.vector.tensor_scalar(out=var[:], in0=astv[:, :, 1],
                                    scalar1=1.0 / cnt, scalar2=None,
                                    op0=ALU.mult)
            msq = pp.tile([128, 2], DT.float32, tag="msq")
            nc.vector.tensor_tensor(out=msq[:], in0=mean[:], in1=mean[:],
                                    op=ALU.mult)
            nc.vector.tensor_tensor(out=var[:], in0=var[:], in1=msq[:],
                                    op=ALU.subtract)
            epsb = pp.tile([128, 1], DT.float32, tag="epsb")
            nc.vector.memset(epsb[:], EPS)
            std = pp.tile([128, 2], DT.float32, tag="std")
            nc.scalar.activation(out=std[:], in_=var[:], func=AF.Sqrt,
                                 bias=epsb[:])
            rstd = pp.tile([128, 2], DT.float32, tag="rstd")
            nc.vector.reciprocal(rstd[:], std[:])
            sc = pp.tile([128, 2], DT.float32, tag="sc")
            nc.vector.tensor_tensor(out=sc[:], in0=rstd[:], in1=gam_sb[:],
                                    op=ALU.mult)
            bb = pp.tile([128, 2], DT.float32, tag="bb")
            nc.vector.tensor_tensor(out=bb[:], in0=mean[:], in1=sc[:],
                                    op=ALU.mult)
            nc.vector.tensor_tensor(out=bb[:], in0=bet_sb[:], in1=bb[:],
                                    op=ALU.subtract)

            # ---------------- P6: affine + LeakyReLU + store ------------------
            SEG = 1024
            for oc in range(2):
                for s in range(NPOS // SEG):
                    seg = slice(oc * NPOS + s * SEG, oc * NPOS + (s + 1) * SEG)
                    a = pp6.tile([128, SEG], DT.float16, tag="p6a")
                    bmul = pp6.tile([128, SEG], DT.float16, tag="p6b")
                    nc.vector.tensor_scalar(out=a[:], in0=out_sb[:, seg],
                                            scalar1=sc[:, oc:oc + 1],
                                            scalar2=bb[:, oc:oc + 1],
                                            op0=ALU.mult, op1=ALU.add)
                    nc.vector.tensor_scalar(out=bmul[:], in0=a[:],
                                            scalar1=float(LEAK),
                                            scalar2=None, op0=ALU.mult)
                    nc.vector.tensor_tensor(out=a[:], in0=a[:],
                                            in1=bmul[:], op=ALU.max)
                    nc.gpsimd.dma_start(out=out_d[oc, :, s * SEG:(s + 1) * SEG],
                                      in_=a[:])

    nc.compile()
    return nc


# ---------------------------------------------------------------------------
# host side
# ---------------------------------------------------------------------------
def prep_in_maps(x, p_w, p_b, w_conv, gamma, beta):
    x = np.asarray(x, np.float32)
    p_w = np.asarray(p_w, np.float32)
    p_b = np.asarray(p_b, np.float32)
    w_conv = np.asarray(w_conv, np.float32)
    gamma = np.asarray(gamma, np.float32)
    beta = np.asarray(beta, np.float32)

    pwT = np.stack([p_w[:, :, t // 3, t % 3].T for t in range(NT)]) \
        .astype(np.float16)                                      # (9, C, 18)
    wT = np.stack([w_conv[:, :, t // 3, t % 3].T for t in range(NT)]) \
        .astype(np.float16)                                      # (9, C, O)
    gamma2 = np.ascontiguousarray(gamma.reshape(2, 128).T)
    beta2 = np.ascontiguousarray(beta.reshape(2, 128).T)
    identf = np.eye(128, dtype=np.float32)
    identh = np.eye(128, dtype=np.float16)

    rr = np.arange(ROWS, dtype=np.float32)[:, None]
    ww = np.arange(W, dtype=np.float32)[:, None, None]
    pbx, pby = p_b[:NT], p_b[NT:]
    by = np.broadcast_to((1 + ww + DY[None, None, :] + pby[None, None, :]),
                         (W, ROWS, NT)).reshape(W, CH).astype(np.float32)

    in_maps = []
    for core in range(N_CORES):
        bi, half = core // 2, core % 2
        h0 = 64 * half
        # padded fp16 image for this batch, flat row-major [pos, c]
        xp16 = np.zeros((Hp * Hp + 2 * Hp, C), np.float16)
        img = x[bi].transpose(1, 2, 0).astype(np.float16)        # (H, W, C)
        xpv = xp16[:Hp * Hp].reshape(Hp, Hp, C)
        xpv[1:1 + H, 1:1 + W] = img
        # 4-corner unit table: unit q = [xp16[q], xp16[q+130]]
        xpt2 = np.zeros((NUNITS, 2 * C), np.float16)
        xpt2[:Hp * Hp, :C] = xp16[:Hp * Hp]
        xpt2[:Hp * Hp, C:] = xp16[Hp:Hp * Hp + Hp]
        # offset-conv window: rows h0-1 .. h0+64 of the unpadded image,
        # embedded at column 1 of a [66, 130] zero canvas
        xw = np.zeros((C, 66, Hp), np.float16)
        lo, hi = h0 - 1, h0 + 65
        glo, ghi = max(lo, 0), min(hi, H)
        xw[:, glo - lo:glo - lo + (ghi - glo), 1:1 + W] = \
            x[bi, :, glo:ghi].astype(np.float16)
        ax = np.broadcast_to((h0 + 1 + rr + DX[None, :] + pbx[None, :]),
                             (ROWS, NT)).reshape(1, CH)
        ax = np.broadcast_to(ax, (128, CH)).astype(np.float32)
        in_maps.append({
            "xpt2": xpt2.reshape(-1),
            "xwin": np.ascontiguousarray(xw.reshape(C, 66 * Hp)),
            "pwT": pwT, "wT": wT,
            "Ax": np.ascontiguousarray(ax), "By": np.ascontiguousarray(by),
            "gamma2": gamma2, "beta2": beta2,
            "identf": identf, "identh": identh,
        })
    return in_maps


def assemble(results):
    out = np.zeros((B, O, H, W), np.float32)
    for core, om in enumerate(results):
        bi, half = core // 2, core % 2
        h0 = 64 * half
        oc = np.asarray(om["out"]).astype(np.float32).reshape(O, ROWS, W)
        out[bi, :, h0:h0 + 64, :] = oc
    return out


_NC_CACHE = {}


def _get_nc(with_collective=True):
    key = with_collective
    if key not in _NC_CACHE:
        _NC_CACHE[key] = build_kernel(with_collective)
    return _NC_CACHE[key]


def kernel(**inputs):
    from concourse.bass_utils import run_bass_kernel_spmd
    nc = _get_nc(True)
    in_maps = prep_in_maps(**inputs)
    res = run_bass_kernel_spmd(nc, in_maps, core_ids=list(range(N_CORES)))
    return assemble(res.results)


if __name__ == "__main__":
    build_kernel(False)
    print("build ok")


# revision 43
# speedup vs baseline: 1.0284x; 1.0284x over previous
"""Deformable-Conv (DCNv1) + SyncBN + LeakyReLU Trainium2 kernel, v2.

Self-contained: shards the full inputs over 8 NeuronCores (data-parallel over
(batch, row-half); BN stats all-reduced on-device), runs one SPMD Bass/Tile
kernel via run_bass_kernel_spmd, and reassembles the full output.

v2 dataflow (per core, 64 output rows x 128 cols, 256 out-ch):
  HOST  build xpt2: "4-corner unit" table in DRAM. Unit q holds the fp16
        channel vectors of padded positions q and q+130 (vertical pair).
        One 1KB gather element = units (q, q+1) = all four bilinear corners.
  P1    offset conv (9 accumulating fp16 matmuls per 4-row group) + PE
        transposes -> offsets in [w-partition, (row, tap)] layout.
  P2    elementwise chain (DVE fp32): bilinear indices + border-folded weights.
  P3    wrapped int16 index build for dma_gather (double PE-transpose trick),
        single index stream (one descriptor per position*tap).
  P4    per (8-row group, tap): one dma_gather (1024 idx, 1KB elems; fits the
        1024-descriptor SWDGE ring); per (tap, row) the 4-term bilinear
        combine runs as
          t = wlt*LT (DVE ts in 4x perf mode)
          psum += T(t) + RT^T*diag(wrt) + LB^T*diag(wlb) + RB^T*diag(wrb)
        where the diag(w) matrices are built by DVE 4x tensor_scalar on the
        identity and the four PE matmuls accumulate scale+transpose+sum in
        one PSUM group. Produce (DVE) runs two taps ahead of consume (PE).
        ACT evicts psum -> fp16 rhs; fp16 matmuls accumulate over taps in
        PSUM; ACT copy-with-accum + Square for BN stats.
  P5    BN stats AllReduce (2x256 floats) + scale/bias solve.
  P6    fused affine+LeakyReLU via max(a, 0.1*a) (DVE) + fp16 store.
"""
import sys

sys.path.insert(0, "/opt/trn_rl_repo")

import numpy as np

import concourse.bacc as bacc
import concourse.mybir as mybir
from concourse import tile
from concourse.ap import AP

ALU = mybir.AluOpType
DT = mybir.dt
AF = mybir.ActivationFunctionType

N_CORES = 8
B, C, O, H, W = 4, 128, 256, 128, 128
KS, NT = 3, 9
ROWS = 64                 # output rows per core
NG2, R8 = 8, 8            # main loop: 8 groups of 8 rows
Hp = H + 2                # 130
NUNITS = 17152            # 130*130 = 16900 units + zero tail
NPOS = ROWS * W           # 8192
EPS = 1e-5
LEAK = 0.1
MAGIC = float(3 << 22)    # 1.5 * 2^23: fp32 round-to-int magic
CH = ROWS * NT            # 576: elementwise-chain free size
CW = NG2 * NT * R8        # 576: index-matrix columns (one per (g2, tap, row))

DX = np.repeat(np.arange(-1, 2), 3).astype(np.float32)
DY = np.tile(np.arange(-1, 2), 3).astype(np.float32)


def build_kernel(with_collective=True):
    nc = bacc.Bacc("TRN2", target_bir_lowering=False)

    # ---- I/O ----
    xpt2_d = nc.dram_tensor("xpt2", [NUNITS * 2 * C], DT.float16,
                            kind="ExternalInput")
    xwin_d = nc.dram_tensor("xwin", [C, 66 * Hp], DT.float16,
                            kind="ExternalInput")
    pwT_d = nc.dram_tensor("pwT", [NT, C, 2 * NT], DT.float16,
                           kind="ExternalInput")
    wT_d = nc.dram_tensor("wT", [NT, C, O], DT.float16, kind="ExternalInput")
    ax_d = nc.dram_tensor("Ax", [128, CH], DT.float32, kind="ExternalInput")
    by_d = nc.dram_tensor("By", [128, CH], DT.float32, kind="ExternalInput")
    gam_d = nc.dram_tensor("gamma2", [128, 2], DT.float32, kind="ExternalInput")
    bet_d = nc.dram_tensor("beta2", [128, 2], DT.float32, kind="ExternalInput")
    idf_d = nc.dram_tensor("identf", [128, 128], DT.float32,
                           kind="ExternalInput")
    idh_d = nc.dram_tensor("identh", [128, 128], DT.float16,
                           kind="ExternalInput")

    out_d = nc.dram_tensor("out", [2, 128, NPOS], DT.float16,
                           kind="ExternalOutput")

    cc_in = nc.dram_tensor("cc_in", [128, 4], DT.float32)
    cc_out = nc.dram_tensor("cc_out", [128, 4], DT.float32)

    taps = [(ky, kx) for ky in range(3) for kx in range(3)]

    with tile.TileContext(nc) as tc:
        with tc.tile_pool(name="pp", bufs=1) as pp, \
             tc.tile_pool(name="pbig", bufs=1) as pbig, \
             tc.tile_pool(name="pch", bufs=10) as pch, \
             tc.tile_pool(name="po", bufs=2) as po, \
             tc.tile_pool(name="pw4", bufs=1) as pw4, \
             tc.tile_pool(name="pg", bufs=6) as pg, \
             tc.tile_pool(name="pst", bufs=3) as pst, \
             tc.tile_pool(name="pdg", bufs=14) as pdg, \
             tc.tile_pool(name="prh", bufs=3) as prh, \
             tc.tile_pool(name="pp6", bufs=2) as pp6, \
             tc.tile_pool(name="pps", bufs=2, space="PSUM") as pps, \
             tc.tile_pool(name="ppre", bufs=1, space="PSUM") as ppre, \
             tc.tile_pool(name="ppacc", bufs=2, space="PSUM") as ppacc:

            # ---------------- constants ----------------
            pw_sb = pp.tile([C, NT * 2 * NT], DT.float16, tag="pw")
            nc.sync.dma_start(pw_sb[:].rearrange("c (t m) -> c t m", m=2 * NT),
                              pwT_d[:].transpose([1, 0, 2]))
            wt_sb = pp.tile([C, NT * O], DT.float16, tag="wt")
            nc.sync.dma_start(wt_sb[:].rearrange("c (t o) -> c t o", o=O),
                              wT_d[:].transpose([1, 0, 2]))
            ax_sb = pp.tile([128, CH], DT.float32, tag="ax")
            nc.sync.dma_start(ax_sb[:], ax_d[:])
            by_sb = pp.tile([128, CH], DT.float32, tag="by")
            nc.sync.dma_start(by_sb[:], by_d[:])
            gam_sb = pp.tile([128, 2], DT.float32, tag="gam")
            nc.sync.dma_start(gam_sb[:], gam_d[:])
            bet_sb = pp.tile([128, 2], DT.float32, tag="bet")
            nc.sync.dma_start(bet_sb[:], bet_d[:])
            idf = pp.tile([128, 128], DT.float32, tag="idf")
            nc.sync.dma_start(idf[:], idf_d[:])
            idh = pp.tile([128, 128], DT.float16, tag="idh")
            nc.sync.dma_start(idh[:], idh_d[:])

            xwin = pbig.tile([C, 66 * Hp], DT.float16, tag="xwin")
            nc.sync.dma_start(xwin[:], xwin_d[:])

            # PE p-state warmup: keep the tensor engine continuously busy so
            # the offset conv runs at full clock.
            wrm = ppre.tile([128, 128], DT.float16, tag="ptr", name="wrm")
            for _ in range(24):
                nc.tensor.transpose(out=wrm[:], in_=idh[:], identity=idh[:])

            # ---------------- P1-P4: pipelined per g2-pair -------------------
            # Prologue work (offset conv, chain, wrapped-index build) is
            # emitted per pair of 8-row groups so the first gather issues
            # after ~1/4 of the prologue; later pairs overlap the gathers.
            offT = pw4.tile([128, ROWS * 2 * NT], DT.float32, tag="offT")
            pwr = pw_sb[:].rearrange("c (t m) -> c t m", m=2 * NT)
            offv = offT[:].rearrange("p (r m) -> p r m", m=2 * NT)
            wlt = pw4.tile([128, CH], DT.float32, tag="wlt")
            wlb = pw4.tile([128, CH], DT.float32, tag="wlb")
            wrt = pw4.tile([128, CH], DT.float32, tag="wrt")
            wrb = pw4.tile([128, CH], DT.float32, tag="wrb")
            cmat = pw4.tile([128, CW], DT.float32, tag="cmat")
            wrap = pw4.tile([128, CW * 8], DT.int16, tag="wrap")

            PC = 144  # chain/index columns per pair (16 rows x 9 taps)

            def emit_pair(k):
                # --- offset conv for rows 16k .. 16k+16 ---
                for g2l in range(2):
                    g2 = 2 * k + g2l
                    ps_tr = ppre.tile([128, PC], DT.float32, tag="ptr",
                                      name="ps_tr")
                    for q in range(2):
                        r0 = g2 * 8 + q * 4
                        ps_off = ppre.tile([2 * NT, 512], DT.float32,
                                           tag="poff", name="ps_off")
                        for t, (ky, kx) in enumerate(taps):
                            base = (r0 + ky) * Hp + kx
                            rhs = AP(xwin.tensor, xwin[:].offset + base,
                                     [xwin[:].ap[0], [Hp, 4], [1, W]])
                            nc.tensor.matmul(ps_off[:], lhsT=pwr[:, t],
                                             rhs=rhs,
                                             start=(t == 0), stop=(t == 8))
                        offc = po.tile([2 * NT, 512], DT.float32, tag="offc",
                                        name="offc")
                        nc.scalar.copy(offc[:], ps_off[:])
                        for r4 in range(4):
                            osl = slice((q * 4 + r4) * 2 * NT,
                                        (q * 4 + r4 + 1) * 2 * NT)
                            nc.tensor.transpose(
                                out=ps_tr[:, osl],
                                in_=offc[:, r4 * 128:(r4 + 1) * 128],
                                identity=idf[:2 * NT, :2 * NT])
                    nc.vector.tensor_copy(
                        offT[:, g2 * PC:(g2 + 1) * PC], ps_tr[:])

                # --- elementwise chain for this pair ---
                cs = slice(k * PC, (k + 1) * PC)

                def cht():
                    return pch.tile([128, PC], DT.float32, tag="ch",
                                    name="cht")

                ox = cht()
                nc.vector.tensor_copy(
                    ox[:].rearrange("p (r n) -> p r n", n=NT),
                    offv[:, 16 * k:16 * k + 16, 0:NT])
                oy = cht()
                nc.vector.tensor_copy(
                    oy[:].rearrange("p (r n) -> p r n", n=NT),
                    offv[:, 16 * k:16 * k + 16, NT:2 * NT])
                px = cht()
                nc.vector.tensor_tensor(out=px[:], in0=ox[:],
                                        in1=ax_sb[:, cs], op=ALU.add)
                py = cht()
                nc.vector.tensor_tensor(out=py[:], in0=oy[:],
                                        in1=by_sb[:, cs], op=ALU.add)

                def floor_(v):
                    fl = cht()
                    nc.vector.tensor_scalar(out=fl[:], in0=v[:],
                                            scalar1=MAGIC, scalar2=MAGIC,
                                            op0=ALU.add, op1=ALU.subtract)
                    g_ = cht()
                    nc.vector.tensor_tensor(out=g_[:], in0=fl[:], in1=v[:],
                                            op=ALU.is_gt)
                    nc.vector.tensor_tensor(out=fl[:], in0=fl[:], in1=g_[:],
                                            op=ALU.subtract)
                    return fl

                fx = floor_(px)
                fy = floor_(py)

                def clip_lo_hi(v):
                    q0 = cht()
                    nc.vector.tensor_scalar(out=q0[:], in0=v[:], scalar1=0.0,
                                            scalar2=129.0, op0=ALU.max,
                                            op1=ALU.min)
                    q1 = cht()
                    nc.vector.tensor_scalar(out=q1[:], in0=v[:], scalar1=-1.0,
                                            scalar2=1.0, op0=ALU.max,
                                            op1=ALU.add)
                    nc.vector.tensor_scalar(out=q1[:], in0=q1[:],
                                            scalar1=129.0, scalar2=None,
                                            op0=ALU.min)
                    return q0, q1

                qltx, qrbx = clip_lo_hi(fx)
                qlty, qrby = clip_lo_hi(fy)
                pcx = cht()
                nc.vector.tensor_scalar(out=pcx[:], in0=px[:], scalar1=0.0,
                                        scalar2=129.0, op0=ALU.max,
                                        op1=ALU.min)
                pcy = cht()
                nc.vector.tensor_scalar(out=pcy[:], in0=py[:], scalar1=0.0,
                                        scalar2=129.0, op0=ALU.max,
                                        op1=ALU.min)

                def weights(qlt, qrb, pc):
                    a0 = cht()
                    nc.vector.scalar_tensor_tensor(out=a0[:], in0=qlt[:],
                                                   scalar=1.0, in1=pc[:],
                                                   op0=ALU.add,
                                                   op1=ALU.subtract)
                    a1 = cht()
                    nc.vector.scalar_tensor_tensor(out=a1[:], in0=pc[:],
                                                   scalar=1.0, in1=qrb[:],
                                                   op0=ALU.add,
                                                   op1=ALU.subtract)
                    eq = cht()
                    nc.vector.tensor_tensor(out=eq[:], in0=qrb[:],
                                            in1=qlt[:], op=ALU.is_equal)
                    t = cht()
                    nc.vector.tensor_tensor(out=t[:], in0=eq[:], in1=a1[:],
                                            op=ALU.mult)
                    nc.vector.tensor_tensor(out=a0[:], in0=a0[:], in1=t[:],
                                            op=ALU.add)
                    nc.vector.tensor_scalar(out=eq[:], in0=eq[:],
                                            scalar1=-1.0, scalar2=1.0,
                                            op0=ALU.mult, op1=ALU.add)
                    nc.vector.tensor_tensor(out=a1[:], in0=a1[:], in1=eq[:],
                                            op=ALU.mult)
                    return a0, a1

                a0, a1 = weights(qltx, qrbx, pcx)
                b0, b1 = weights(qlty, qrby, pcy)

                nc.vector.tensor_tensor(out=wlt[:, cs], in0=a0[:], in1=b0[:],
                                        op=ALU.mult)
                nc.vector.tensor_tensor(out=wlb[:, cs], in0=a0[:], in1=b1[:],
                                        op=ALU.mult)
                nc.vector.tensor_tensor(out=wrt[:, cs], in0=a1[:], in1=b0[:],
                                        op=ALU.mult)
                nc.vector.tensor_tensor(out=wrb[:, cs], in0=a1[:], in1=b1[:],
                                        op=ALU.mult)

                idx0 = cht()
                nc.vector.scalar_tensor_tensor(out=idx0[:], in0=qltx[:],
                                               scalar=130.0, in1=qlty[:],
                                               op0=ALU.mult, op1=ALU.add)

                cview = cmat[:, cs].rearrange("p (g n j) -> p g n j",
                                              g=2, n=NT)
                sview = idx0[:].rearrange("p (g j n) -> p g j n", g=2, j=R8)
                nc.vector.tensor_copy(cview, sview.transpose([0, 1, 3, 2]))

                # --- wrapped int16 index build for this pair ---
                for lo_l, hi_l in ((0, 128), (128, PC)):
                    lo = k * PC + lo_l
                    cksz = hi_l - lo_l
                    ps = ppre.tile([128, 128], DT.float32, tag="ptr",
                                   name="psT2")
                    nc.tensor.transpose(out=ps[:cksz, :],
                                        in_=cmat[:, lo:lo + cksz],
                                        identity=idf[:])
                    tsb = po.tile([128, 128], DT.float32, tag="tsb",
                                   name="tsb")
                    nc.scalar.copy(tsb[:cksz, :], ps[:cksz, :])
                    for a in range(8):
                        wa = ppre.tile([16, 128], DT.float32, tag="ptr",
                                       name="wa")
                        nc.tensor.transpose(
                            out=wa[:, :cksz],
                            in_=tsb[:cksz, 16 * a:16 * a + 16],
                            identity=idf[:cksz, :cksz])
                        dstv = AP(wrap.tensor, wrap[:].offset
                                  + (lo // 16) * 128 + a,
                                  [[wrap[:].ap[0][0], 16],
                                   [128, cksz // 16], [8, 16]])
                        nc.vector.tensor_copy(
                            dstv,
                            wa[:, :cksz].rearrange("p (c j) -> p c j", j=16))
                for cgrp in range(1, 8):
                    wsl = slice(k * PC * 8, (k + 1) * PC * 8)
                    nc.gpsimd.dma_start(
                        out=wrap[16 * cgrp:16 * (cgrp + 1), wsl],
                        in_=wrap[0:16, wsl])

            # ---------------- P4: gather + combine + matmul ------------------
            src_ap = AP(xpt2_d, 0, [[2 * C, NUNITS - 1], [1, 4 * C]])
            out_sb = pbig.tile([128, 2 * NPOS], DT.float16, tag="osb")
            sums = pp.tile([128, 16], DT.float32, tag="sums")
            sqs = pp.tile([128, 16], DT.float32, tag="sqs")
            junk = pp.tile([128, R8 * W], DT.float32, tag="junk")
            wtv = wt_sb[:].rearrange("c (t o) -> c t o", o=O)

            emit_pair(0)
            for g2 in range(NG2):
                pacc = [ppacc.tile([128, R8 * W], DT.float32, tag="acc",
                                   name=f"pacc{i}") for i in range(2)]
                g_ts = []
                for n in range(NT):
                    g_t = pg.tile([128, 8, 4 * C], DT.float16, tag="gt",
                                  name="g_t")
                    fbase = g2 * CW + n * 64
                    nc.gpsimd.dma_gather(
                        out_ap=g_t[:], in_ap=src_ap,
                        idxs_ap=wrap[:, fbase:fbase + 64],
                        num_idxs=8 * 128, num_idxs_reg=8 * 128,
                        elem_size=4 * C, elem_step=2 * C)
                    g_ts.append(g_t)

                def produce(n):
                    g_t = g_ts[n]
                    ta = pst.tile([128, R8 * W], DT.float16, tag="ta")
                    diags = []
                    # diag builds first: they depend only on the weights, so
                    # DVE keeps working while the gather for tap n lands.
                    for jj in range(R8):
                        col = (g2 * R8 + jj) * NT + n
                        drt = pdg.tile([128, 128], DT.float16, tag="drt",
                                       name="drt")
                        nc.vector.tensor_scalar(
                            out=drt[:], in0=idh[:],
                            scalar1=wrt[:, col:col + 1], scalar2=None,
                            op0=ALU.mult)
                        dlb = pdg.tile([128, 128], DT.float16, tag="dlb",
                                       name="dlb")
                        if (jj + n) % 2 == 0:
                            nc.vector.tensor_scalar(
                                out=dlb[:], in0=idh[:],
                                scalar1=wlb[:, col:col + 1], scalar2=None,
                                op0=ALU.mult)
                        else:
                            nc.scalar.activation(
                                out=dlb[:], in_=idh[:], func=AF.Copy,
                                scale=wlb[:, col:col + 1])
                        drb = pdg.tile([128, 128], DT.float16, tag="drb",
                                       name="drb")
                        nc.vector.tensor_scalar(
                            out=drb[:], in0=idh[:],
                            scalar1=wrb[:, col:col + 1],
                            scalar2=None, op0=ALU.mult)
                        diags.append((drt, dlb, drb))
                    for jj in range(R8):
                        col = (g2 * R8 + jj) * NT + n
                        sl = slice(jj * 128, (jj + 1) * 128)
                        nc.vector.tensor_scalar(
                            out=ta[:, sl], in0=g_t[:, jj, 0:C],
                            scalar1=wlt[:, col:col + 1], scalar2=None,
                            op0=ALU.mult)
                    return (g_t, n, ta, diags)

                def consume(st):
                    g_t, n, ta, diags = st
                    rhs16 = prh.tile([128, R8 * W], DT.float16, tag="rhs16")
                    for hh4 in range(2):
                        ps_t = pps.tile([128, 512], DT.float32, tag="tp",
                                        name="ps_t")
                        for jj in range(hh4 * 4, hh4 * 4 + 4):
                            ck = jj
                            sl = slice(jj * 128, (jj + 1) * 128)
                            psl = slice((jj % 4) * 128, (jj % 4 + 1) * 128)
                            drt, dlb, drb = diags[jj]
                            nc.tensor.matmul(ps_t[:, psl], lhsT=ta[:, sl],
                                             rhs=idh[:],
                                             start=True, stop=False)
                            nc.tensor.matmul(ps_t[:, psl],
                                             lhsT=g_t[:, ck, C:2 * C],
                                             rhs=drt[:], start=False,
                                             stop=False)
                            nc.tensor.matmul(ps_t[:, psl],
                                             lhsT=g_t[:, ck, 2 * C:3 * C],
                                             rhs=dlb[:], start=False,
                                             stop=False)
                            nc.tensor.matmul(ps_t[:, psl],
                                             lhsT=g_t[:, ck, 3 * C:4 * C],
                                             rhs=drb[:], start=False,
                                             stop=True)
                        hsl = slice(hh4 * 512, (hh4 + 1) * 512)
                        nc.scalar.copy(rhs16[:, hsl], ps_t[:])
                    for oc in range(2):
                        for hh in range(2):
                            sl = slice(hh * 512, (hh + 1) * 512)
                            nc.tensor.matmul(
                                pacc[oc][:, sl],
                                lhsT=wtv[:, n, oc * 128:(oc + 1) * 128],
                                rhs=rhs16[:, sl],
                                start=(n == 0), stop=(n == 8))

                pend = []
                for n in range(NT):
                    pend.append(produce(n))
                    if len(pend) >= 4:
                        consume(pend.pop(0))
                if g2 == 0:
                    emit_pair(1)
                if g2 == 2:
                    emit_pair(2)
                if g2 == 4:
                    emit_pair(3)
                while pend:
                    consume(pend.pop(0))
                for oc in range(2):
                    seg = slice(oc * NPOS + g2 * R8 * W,
                                oc * NPOS + (g2 + 1) * R8 * W)
                    nc.scalar.activation(
                        out=out_sb[:, seg], in_=pacc[oc][:], func=AF.Copy,
                        accum_out=sums[:, oc * 8 + g2:oc * 8 + g2 + 1])
                    nc.scalar.activation(
                        out=junk[:], in_=pacc[oc][:], func=AF.Square,
                        accum_out=sqs[:, oc * 8 + g2:oc * 8 + g2 + 1])

            # ---------------- P5: BN stats + collective ----------------------
            stats = pp.tile([128, 4], DT.float32, tag="stats")
            nc.vector.tensor_reduce(out=stats[:, 0:1], in_=sums[:, 0:8],
                                    axis=mybir.AxisListType.X, op=ALU.add)
            nc.vector.tensor_reduce(out=stats[:, 1:2], in_=sqs[:, 0:8],
                                    axis=mybir.AxisListType.X, op=ALU.add)
            nc.vector.tensor_reduce(out=stats[:, 2:3], in_=sums[:, 8:16],
                                    axis=mybir.AxisListType.X, op=ALU.add)
            nc.vector.tensor_reduce(out=stats[:, 3:4], in_=sqs[:, 8:16],
                                    axis=mybir.AxisListType.X, op=ALU.add)
            d1 = nc.sync.dma_start(out=cc_in[:], in_=stats[:])
            from concourse.tile_rust import add_dep_helper
            if with_collective:
                cci = nc.gpsimd.collective_compute(
                    "AllReduce", ALU.add,
                    replica_groups=[list(range(N_CORES))],
                    ins=[cc_in[:].opt()], outs=[cc_out[:].opt()])
            else:
                cci = nc.sync.dma_start(out=cc_out[:], in_=cc_in[:])
            add_dep_helper(cci.ins, d1.ins, sync=True, reason="cc after stats")
            ast = pp.tile([128, 4], DT.float32, tag="ast")
            d2 = nc.sync.dma_start(out=ast[:], in_=cc_out[:])
            add_dep_helper(d2.ins, cci.ins, sync=True, reason="readback")

            astv = ast[:].rearrange("p (a b) -> p a b", b=2)
            cnt = float(B * H * W)
            mean = pp.tile([128, 2], DT.float32, tag="mean")
            nc.vector.tensor_scalar(out=mean[:], in0=astv[:, :, 0],
                                    scalar1=1.0 / cnt, scalar2=None,
                                    op0=ALU.mult)
            var = pp.tile([128, 2], DT.float32, tag="var")
            nc.vector.tensor_scalar(out=var[:], in0=astv[:, :, 1],
                                    scalar1=1.0 / cnt, scalar2=None,
                                    op0=ALU.mult)
            msq = pp.tile([128, 2], DT.float32, tag="msq")
            nc.vector.tensor_tensor(out=msq[:], in0=mean[:], in1=mean[:],
                                    op=ALU.mult)
            nc.vector.tensor_tensor(out=var[:], in0=var[:], in1=msq[:],
                                    op=ALU.subtract)
            epsb = pp.tile([128, 1], DT.float32, tag="epsb")
            nc.vector.memset(epsb[:], EPS)
            std = pp.tile([128, 2], DT.float32, tag="std")
            nc.scalar.activation(out=std[:], in_=var[:], func=AF.Sqrt,
                                 bias=epsb[:])
            rstd = pp.tile([128, 2], DT.float32, tag="rstd")
            nc.vector.reciprocal(rstd[:], std[:])
            sc = pp.tile([128, 2], DT.float32, tag="sc")
            nc.vector.tensor_tensor(out=sc[:], in0=rstd[:], in1=gam_sb[:],
                                    op=ALU.mult)
            bb = pp.tile([128, 2], DT.float32, tag="bb")
            nc.vector.tensor_tensor(out=bb[:], in0=mean[:], in1=sc[:],
                                    op=ALU.mult)
            nc.vector.tensor_tensor(out=bb[:], in0=bet_sb[:], in1=bb[:],
                                    op=ALU.subtract)

            # ---------------- P6: affine + LeakyReLU + store ------------------
            SEG = 1024
            for oc in range(2):
                for s in range(NPOS // SEG):
                    seg = slice(oc * NPOS + s * SEG, oc * NPOS + (s + 1) * SEG)
                    a = pst.tile([128, SEG], DT.float16, tag="ta",
                                 name="p6a")
                    bmul = pp6.tile([128, SEG], DT.float16,
                                    tag="p6a" if s % 2 == 0 else "p6b",
                                    name="p6b")
                    nc.vector.tensor_scalar(out=a[:], in0=out_sb[:, seg],
                                            scalar1=sc[:, oc:oc + 1],
                                            scalar2=bb[:, oc:oc + 1],
                                            op0=ALU.mult, op1=ALU.add)
                    nc.vector.tensor_scalar(out=bmul[:], in0=a[:],
                                            scalar1=float(LEAK),
                                            scalar2=None, op0=ALU.mult)
                    nc.vector.tensor_tensor(out=a[:], in0=a[:],
                                            in1=bmul[:], op=ALU.max)
                    nc.gpsimd.dma_start(out=out_d[oc, :, s * SEG:(s + 1) * SEG],
                                      in_=a[:])

    nc.compile()
    return nc


# ---------------------------------------------------------------------------
# host side
# ---------------------------------------------------------------------------
def prep_in_maps(x, p_w, p_b, w_conv, gamma, beta):
    x = np.asarray(x, np.float32)
    p_w = np.asarray(p_w, np.float32)
    p_b = np.asarray(p_b, np.float32)
    w_conv = np.asarray(w_conv, np.float32)
    gamma = np.asarray(gamma, np.float32)
    beta = np.asarray(beta, np.float32)

    pwT = np.stack([p_w[:, :, t // 3, t % 3].T for t in range(NT)]) \
        .astype(np.float16)                                      # (9, C, 18)
    wT = np.stack([w_conv[:, :, t // 3, t % 3].T for t in range(NT)]) \
        .astype(np.float16)                                      # (9, C, O)
    gamma2 = np.ascontiguousarray(gamma.reshape(2, 128).T)
    beta2 = np.ascontiguousarray(beta.reshape(2, 128).T)
    identf = np.eye(128, dtype=np.float32)
    identh = np.eye(128, dtype=np.float16)

    rr = np.arange(ROWS, dtype=np.float32)[:, None]
    ww = np.arange(W, dtype=np.float32)[:, None, None]
    pbx, pby = p_b[:NT], p_b[NT:]
    by = np.broadcast_to((1 + ww + DY[None, None, :] + pby[None, None, :]),
                         (W, ROWS, NT)).reshape(W, CH).astype(np.float32)

    in_maps = []
    for core in range(N_CORES):
        bi, half = core // 2, core % 2
        h0 = 64 * half
        # padded fp16 image for this batch, flat row-major [pos, c]
        xp16 = np.zeros((Hp * Hp + 2 * Hp, C), np.float16)
        img = x[bi].transpose(1, 2, 0).astype(np.float16)        # (H, W, C)
        xpv = xp16[:Hp * Hp].reshape(Hp, Hp, C)
        xpv[1:1 + H, 1:1 + W] = img
        # 4-corner unit table: unit q = [xp16[q], xp16[q+130]]
        xpt2 = np.zeros((NUNITS, 2 * C), np.float16)
        xpt2[:Hp * Hp, :C] = xp16[:Hp * Hp]
        xpt2[:Hp * Hp, C:] = xp16[Hp:Hp * Hp + Hp]
        # offset-conv window: rows h0-1 .. h0+64 of the unpadded image,
        # embedded at column 1 of a [66, 130] zero canvas
        xw = np.zeros((C, 66, Hp), np.float16)
        lo, hi = h0 - 1, h0 + 65
        glo, ghi = max(lo, 0), min(hi, H)
        xw[:, glo - lo:glo - lo + (ghi - glo), 1:1 + W] = \
            x[bi, :, glo:ghi].astype(np.float16)
        ax = np.broadcast_to((h0 + 1 + rr + DX[None, :] + pbx[None, :]),
                             (ROWS, NT)).reshape(1, CH)
        ax = np.broadcast_to(ax, (128, CH)).astype(np.float32)
        in_maps.append({
            "xpt2": xpt2.reshape(-1),
            "xwin": np.ascontiguousarray(xw.reshape(C, 66 * Hp)),
            "pwT": pwT, "wT": wT,
            "Ax": np.ascontiguousarray(ax), "By": np.ascontiguousarray(by),
            "gamma2": gamma2, "beta2": beta2,
            "identf": identf, "identh": identh,
        })
    return in_maps


def assemble(results):
    out = np.zeros((B, O, H, W), np.float32)
    for core, om in enumerate(results):
        bi, half = core // 2, core % 2
        h0 = 64 * half
        oc = np.asarray(om["out"]).astype(np.float32).reshape(O, ROWS, W)
        out[bi, :, h0:h0 + 64, :] = oc
    return out


_NC_CACHE = {}


def _get_nc(with_collective=True):
    key = with_collective
    if key not in _NC_CACHE:
        _NC_CACHE[key] = build_kernel(with_collective)
    return _NC_CACHE[key]


def kernel(**inputs):
    from concourse.bass_utils import run_bass_kernel_spmd
    nc = _get_nc(True)
    in_maps = prep_in_maps(**inputs)
    res = run_bass_kernel_spmd(nc, in_maps, core_ids=list(range(N_CORES)))
    return assemble(res.results)


if __name__ == "__main__":
    build_kernel(False)
    print("build ok")
